# revision 60
# baseline (speedup 1.0000x reference)
"""Trainium2 Bass kernel for CartesianDensityBlock (GNN message passing).

Strategy:
  * Host: sort edges by destination node; greedily pack consecutive nodes
    into "windows" of <=128 nodes and <=640 edges (5 tiles of 128 edge
    slots).  Windows are distributed contiguously across 8 cores, so every
    node's edges live on exactly one core -> no collectives.
  * Device (per window): segment-sum via one-hot matmuls on TensorE
    producing feature-major densities denT [832f, 128n] in PSUM, then
    rotation invariants + MLPs + channel-mix + gating entirely on-chip,
    emitting node-major outputs.
  * Host: scatter per-window rows back to the full [N, ...] outputs.
"""

import os
import sys

import numpy as np

for _p in ("/opt/trn_rl_repo",):
    if _p not in sys.path:
        sys.path.insert(0, _p)

P = 128
TPW = 5                # edge tiles per window
CAP = TPW * P          # max edges per window
NCORES = 8
FT = 832               # 64 + 3*64 + 9*64 features per edge
INV_SQRT_DEG = 1.0 / 50.0 ** 0.5

# set KERNEL_PROFILE=1 in the environment to capture an NTFF profile
LAST_EXEC_NS = None
LAST_RESULTS = None

_AXON_SO = "/opt/axon/libaxon_pjrt.so"


def _install_ntff_hook():
    """Provide antenv.axon_hooks (absent in this image) so that
    run_bass_kernel_spmd(trace=True) can capture NTFF profiles."""
    import types
    import ctypes
    import contextlib

    try:
        from antenv.axon_hooks import get_axon_ntff_profile_hook  # noqa
        return
    except ImportError:
        pass
    if not os.path.exists(_AXON_SO):
        return

    lib = ctypes.CDLL(_AXON_SO)
    if not hasattr(lib, "axon_start_nrt_profile"):
        return
    lib.axon_start_nrt_profile.argtypes = [
        ctypes.POINTER(ctypes.c_int64), ctypes.c_size_t]
    lib.axon_start_nrt_profile.restype = ctypes.c_int64
    lib.axon_stop_nrt_profile.argtypes = [ctypes.c_char_p]
    lib.axon_stop_nrt_profile.restype = ctypes.c_int64

    @contextlib.contextmanager
    def _hook(output_dir, device_ids):
        import jax
        jax.devices()
        if device_ids:
            ids = (ctypes.c_int64 * len(device_ids))(*device_ids)
            rc = lib.axon_start_nrt_profile(ids, len(device_ids))
        else:
            rc = lib.axon_start_nrt_profile(None, 0)
        if rc != 0:
            raise RuntimeError(f"axon_start_nrt_profile rc={rc}")
        try:
            yield
        finally:
            n = lib.axon_stop_nrt_profile(str(output_dir).encode())
            print(f"profile: {n} file(s) written to {output_dir}",
                  file=sys.stderr)

    mod = types.ModuleType("antenv.axon_hooks")
    mod._hook = _hook
    mod.get_axon_ntff_profile_hook = lambda: _hook
    mod.set_axon_ntff_profile_hook = lambda h: None
    import antenv
    antenv.axon_hooks = mod
    sys.modules["antenv.axon_hooks"] = mod


def _pack_windows(idx, num_nodes):
    """Greedy packing of consecutive (sorted) nodes into windows."""
    counts = np.bincount(idx, minlength=num_nodes)
    assert counts.max() <= CAP, "node degree exceeds window capacity"
    starts, ncnt, ecnt = [], [], []
    n0 = 0
    while n0 < num_nodes:
        hi = min(n0 + P, num_nodes)
        c = np.cumsum(counts[n0:hi])
        k = int(np.searchsorted(c, CAP, side="right"))
        k = max(k, 1)
        starts.append(n0)
        ncnt.append(k)
        ecnt.append(int(c[k - 1]))
        n0 += k
    return (np.asarray(starts, np.int64), np.asarray(ncnt, np.int64),
            np.asarray(ecnt, np.int64))


def _build_program(W, msg_dt_np, stage=99):
    import concourse.bacc as bacc
    import concourse.mybir as mybir
    import concourse.tile as tile

    dt = mybir.dt
    f32 = dt.float32
    mdt = dt.from_np(np.dtype(msg_dt_np))
    A = mybir.AluOpType
    AF = mybir.ActivationFunctionType

    nc = bacc.Bacc("TRN2", target_bir_lowering=False, debug=False)

    # msgs layout: [W*128 rows, TPW*FT] — row (w, p) holds the feature
    # vectors of the 5 edges that land on partition p in window w, so a
    # whole window loads as ONE DMA with one descriptor per partition.
    msgs = nc.dram_tensor("msgs", [W * P, TPW * FT], mdt,
                          kind="ExternalInput")
    lidx = nc.dram_tensor("lidx", [P, W * TPW], f32, kind="ExternalInput")
    iota = nc.dram_tensor("iota", [P, P], mdt, kind="ExternalInput")
    ws1t0 = nc.dram_tensor("ws1t0", [64, 64], mdt, kind="ExternalInput")
    ws1t1 = nc.dram_tensor("ws1t1", [64, 64], mdt, kind="ExternalInput")
    ws1t2 = nc.dram_tensor("ws1t2", [64, 64], mdt, kind="ExternalInput")
    fold = nc.dram_tensor("fold", [128, 64], mdt, kind="ExternalInput")
    selhi = nc.dram_tensor("selhi", [128, 64], mdt, kind="ExternalInput")
    id64 = nc.dram_tensor("id64", [64, 64], mdt, kind="ExternalInput")
    ws2 = nc.dram_tensor("ws2", [64, 64], mdt, kind="ExternalInput")
    wg1 = nc.dram_tensor("wg1", [64, 64], mdt, kind="ExternalInput")
    wg2 = nc.dram_tensor("wg2", [64, 128], mdt, kind="ExternalInput")
    wl1 = nc.dram_tensor("wl1", [128, 64], mdt, kind="ExternalInput")
    wl2 = nc.dram_tensor("wl2", [128, 64], mdt, kind="ExternalInput")
    bs1 = nc.dram_tensor("bs1", [64, 1], f32, kind="ExternalInput")
    bg1 = nc.dram_tensor("bg1", [64, 1], f32, kind="ExternalInput")
    bs2c = nc.dram_tensor("bs2c", [64, 1], f32, kind="ExternalInput")
    bs2b = nc.dram_tensor("bs2b", [P, 64], f32, kind="ExternalInput")
    bg2b = nc.dram_tensor("bg2b", [P, 128], f32, kind="ExternalInput")
    eps = nc.dram_tensor("eps", [64, 1], f32, kind="ExternalInput")

    # single merged output: [delta_h0 (64) | delta_h1 (192) | delta_h2
    # (576)] per node row -> one DMA per window.
    outA = nc.dram_tensor("outA", [W * P, FT], f32, kind="ExternalOutput")

    with tile.TileContext(nc) as tc:
        with (
            tc.tile_pool(name="const", bufs=1) as cp,
            tc.tile_pool(name="mpool", bufs=6) as mp,
            tc.tile_pool(name="ohpool", bufs=20) as ohp,
            tc.tile_pool(name="work", bufs=3) as wp,
            tc.tile_pool(name="outp", bufs=4) as op,
            tc.tile_pool(name="pden", bufs=2, space="PSUM") as pden,
            tc.tile_pool(name="pmlp", bufs=4, space="PSUM") as pmlp,
        ):
            def cload(dram, shape, dtype=f32):
                t = cp.tile(shape, dtype, tag=dram.name)
                nc.sync.dma_start(out=t[:], in_=dram[:])
                return t

            iota_t = cload(iota, [P, P], mdt)
            lidx_t = cload(lidx, [P, W * TPW])
            ws1t0_t = cload(ws1t0, [64, 64], mdt)
            ws1t1_t = cload(ws1t1, [64, 64], mdt)
            ws1t2_t = cload(ws1t2, [64, 64], mdt)
            fold_t = cload(fold, [128, 64], mdt)
            selhi_t = cload(selhi, [128, 64], mdt)
            id64_t = cload(id64, [64, 64], mdt)
            ws2_t = cload(ws2, [64, 64], mdt)
            wg1_t = cload(wg1, [64, 64], mdt)
            wg2_t = cload(wg2, [64, 128], mdt)
            wl1_t = cload(wl1, [128, 64], mdt)
            wl2_t = cload(wl2, [128, 64], mdt)
            bs1_t = cload(bs1, [64, 1])
            bg1_t = cload(bg1, [64, 1])
            bs2c_t = cload(bs2c, [64, 1])
            bs2b_t = cload(bs2b, [P, 64])
            bg2b_t = cload(bg2b, [P, 128])
            eps_t = cload(eps, [64, 1])

            def ph1(w):
                # ---------- phase 1: segment-sum into denT (PSUM) ----------
                # chunk-major matmul order: exactly one open accumulation
                # group per PSUM bank at any time.
                pA = pden.tile([P, 512], f32, tag="pA", name=f"pA{w}")
                pB = pden.tile([P, 384], f32, tag="pB", name=f"pB{w}")
                mt = mp.tile([P, TPW * FT], mdt, tag="mt", name=f"mt{w}")
                nc.sync.dma_start(out=mt[:], in_=msgs[w * P:(w + 1) * P, :])
                ohs = []
                for k in range(TPW):
                    g = w * TPW + k
                    oh = ohp.tile([P, P], mdt, tag="oh", name=f"oh{g}")
                    nc.vector.tensor_scalar(
                        oh[:], iota_t[:], lidx_t[:, g:g + 1], None, A.is_equal)
                    ohs.append(oh)
                for c in range(7):
                    lo = c * 128
                    hi = min(lo + 128, FT)
                    m = hi - lo
                    if c < 4:
                        dst = pA[:m, lo:lo + 128]
                    else:
                        dst = pB[:m, (c - 4) * 128:(c - 4) * 128 + 128]
                    for k in range(TPW):
                        nc.tensor.matmul(dst,
                                         lhsT=mt[:, k * FT + lo:k * FT + hi],
                                         rhs=ohs[k][:],
                                         start=(k == 0), stop=(k == TPW - 1))
                return pA, pB

            def ph2(w, pA, pB):
                if stage < 1:
                    return
                # ---------- phase 2: scale + invariants ----------
                s = INV_SQRT_DEG
                sA = wp.tile([P, 512], mdt, tag="sA")
                nc.vector.tensor_scalar_mul(sA[:], pA[:], s)
                sB1 = wp.tile([P, 256], mdt, tag="sB1")
                nc.vector.tensor_scalar_mul(sB1[:], pB[:, 0:256], s)
                sB2 = wp.tile([64, 128], mdt, tag="sB2")
                nc.vector.tensor_scalar_mul(sB2[:], pB[0:64, 256:384], s)

                if stage < 2:
                    o0 = op.tile([P, 64], f32, tag="o0")
                    nc.vector.tensor_copy(out=o0[:], in_=sA[:, 0:64])
                    nc.sync.dma_start(out=outA[w * P:(w + 1) * P, 0:64],
                                      in_=o0[:])
                    return
                sqA = wp.tile([P, 512], mdt, tag="sqA")
                nc.vector.tensor_tensor(sqA[:], sA[:], sA[:], op=A.mult)
                sqB1 = wp.tile([P, 256], mdt, tag="sqB1")
                nc.vector.tensor_tensor(sqB1[:], sB1[:], sB1[:], op=A.mult)
                sqB2 = wp.tile([64, 128], mdt, tag="sqB2")
                nc.vector.tensor_tensor(sqB2[:], sB2[:], sB2[:], op=A.mult)

                # inv^2: fold den1/den2 group squares on TensorE with 0/1
                # selection matrices
                pi1 = pmlp.tile([64, 128], f32, tag="pm")
                nc.tensor.matmul(pi1[:], lhsT=selhi_t[:], rhs=sqA[:, 0:128],
                                 start=True, stop=False)
                nc.tensor.matmul(pi1[:], lhsT=fold_t[:], rhs=sqA[:, 128:256],
                                 start=False, stop=True)
                v1 = wp.tile([64, 128], mdt, tag="v1")
                nc.scalar.activation(v1[:], pi1[:], AF.Sqrt,
                                     bias=eps_t[:, 0:1])
                pi2 = pmlp.tile([64, 128], f32, tag="pm")
                nc.tensor.matmul(pi2[:], lhsT=fold_t[:], rhs=sqA[:, 256:384],
                                 start=True, stop=False)
                nc.tensor.matmul(pi2[:], lhsT=fold_t[:], rhs=sqA[:, 384:512],
                                 start=False, stop=False)
                nc.tensor.matmul(pi2[:], lhsT=fold_t[:], rhs=sqB1[:, 0:128],
                                 start=False, stop=False)
                nc.tensor.matmul(pi2[:], lhsT=fold_t[:], rhs=sqB1[:, 128:256],
                                 start=False, stop=False)
                nc.tensor.matmul(pi2[:], lhsT=id64_t[:], rhs=sqB2[:],
                                 start=False, stop=True)
                v2 = wp.tile([64, 128], mdt, tag="v2")
                nc.scalar.activation(v2[:], pi2[:], AF.Sqrt,
                                     bias=eps_t[:, 0:1])

                if stage < 3:
                    o0 = op.tile([P, 64], f32, tag="o0")
                    nc.vector.tensor_copy(out=o0[0:64, :], in_=v1[:, 0:64])
                    nc.vector.tensor_copy(out=o0[64:128, :], in_=v2[:, 0:64])
                    nc.sync.dma_start(out=outA[w * P:(w + 1) * P, 0:64],
                                      in_=o0[:])
                    return
                # ---------- scalar-update MLP (feature-major) ----------
                p1 = pmlp.tile([64, 128], f32, tag="pm")
                nc.tensor.matmul(p1[:], lhsT=ws1t0_t[:], rhs=sA[0:64, 0:128],
                                 start=True, stop=False)
                nc.tensor.matmul(p1[:], lhsT=ws1t1_t[:], rhs=v1[:],
                                 start=False, stop=False)
                nc.tensor.matmul(p1[:], lhsT=ws1t2_t[:], rhs=v2[:],
                                 start=False, stop=True)
                hx = wp.tile([64, 128], mdt, tag="hx")
                nc.vector.tensor_scalar(hx[:], p1[:], bs1_t[:, 0:1], None,
                                        A.add)
                hs = wp.tile([64, 128], mdt, tag="hs")
                nc.scalar.activation(hs[:], p1[:], AF.Sigmoid,
                                     bias=bs1_t[:, 0:1])
                hT = wp.tile([64, 128], mdt, tag="hT")
                nc.vector.tensor_tensor(hT[:], hx[:], hs[:], op=A.mult)

                pd = pmlp.tile([64, 128], f32, tag="pm")
                nc.tensor.matmul(pd[:], lhsT=ws2_t[:], rhs=hT[:],
                                 start=True, stop=True)
                dh0T = wp.tile([64, 128], mdt, tag="dh0T")
                nc.vector.tensor_scalar(dh0T[:], pd[:], bs2c_t[:, 0:1], None,
                                        A.add)

                # node-major delta_h0 -> output
                pn0 = pmlp.tile([P, 64], f32, tag="pm")
                nc.tensor.matmul(pn0[:], lhsT=hT[:], rhs=ws2_t[:],
                                 start=True, stop=True)
                oA = op.tile([P, FT], f32, tag="oA")
                nc.vector.tensor_tensor(oA[:, 0:64], pn0[:], bs2b_t[:],
                                        op=A.add)

                if stage < 4:
                    return
                # ---------- gating scales ----------
                pg = pmlp.tile([64, 128], f32, tag="pm")
                nc.tensor.matmul(pg[:], lhsT=wg1_t[:], rhs=dh0T[:],
                                 start=True, stop=True)
                gx = wp.tile([64, 128], mdt, tag="gx")
                nc.vector.tensor_scalar(gx[:], pg[:], bg1_t[:, 0:1], None,
                                        A.add)
                gs = wp.tile([64, 128], mdt, tag="gs")
                nc.scalar.activation(gs[:], pg[:], AF.Sigmoid,
                                     bias=bg1_t[:, 0:1])
                hgT = wp.tile([64, 128], mdt, tag="hgT")
                nc.vector.tensor_tensor(hgT[:], gx[:], gs[:], op=A.mult)

                ps = pmlp.tile([P, 128], f32, tag="pm")
                nc.tensor.matmul(ps[:], lhsT=hgT[:], rhs=wg2_t[:],
                                 start=True, stop=True)
                scal = wp.tile([P, 128], f32, tag="scal")
                nc.vector.tensor_tensor(scal[:], ps[:], bg2b_t[:], op=A.add)

                if stage < 5:
                    return
                # ---------- delta_h1 / delta_h2 channel mixing ----------
                # PE runs matmuls with disjoint row-groups (lhsT base 0 vs
                # 64) concurrently, so same-bank PSUM writes from different
                # bases are fatal: split outputs into per-base bank slots.
                d1s = (sA[64:128, 0:128], sA[0:64, 128:256],
                       sA[64:128, 128:256])
                d2s = (sA[0:64, 256:384], sA[64:128, 256:384],
                       sA[0:64, 384:512], sA[64:128, 384:512],
                       sB1[0:64, 0:128], sB1[64:128, 0:128],
                       sB1[0:64, 128:256], sB1[64:128, 128:256],
                       sB2[:, :])
                # base-0 sourced groups (d1_1 + even d2) in one bank slot,
                # base-64 sourced (d1_0, d1_2 + odd d2) in another
                pLO = pmlp.tile([P, 384], f32, tag="pm")
                pHI = pmlp.tile([P, 384], f32, tag="pm")
                los = (d1s[1], d2s[0], d2s[2], d2s[4], d2s[6], d2s[8])
                his = (d1s[0], d1s[2], d2s[1], d2s[3], d2s[5], d2s[7])
                for j, dsrc in enumerate(los):
                    wmix = wl2_t if j else wl1_t
                    nc.tensor.matmul(pLO[:, 64 * j:64 * j + 64],
                                     lhsT=dsrc, rhs=wmix[0:64, :],
                                     start=True, stop=True)
                for j, dsrc in enumerate(his):
                    wmix = wl2_t if j >= 2 else wl1_t
                    nc.tensor.matmul(pHI[:, 64 * j:64 * j + 64],
                                     lhsT=dsrc, rhs=wmix[64:128, :],
                                     start=True, stop=True)

                # delta_h1: i=0 -> pHI col0, i=1 -> pLO col0, i=2 -> pHI col1
                d1src = (pHI[:, 0:64], pLO[:, 0:64], pHI[:, 64:128])
                for i in range(3):
                    nc.vector.tensor_tensor(oA[:, 64 + 64 * i:128 + 64 * i],
                                            d1src[i], scal[:, 0:64],
                                            op=A.mult)
                # delta_h2: even k -> pLO col 1+k/2, odd k -> pHI col 2+(k-1)/2
                for k2 in range(9):
                    if k2 % 2 == 0:
                        src = pLO[:, 64 + 32 * k2:128 + 32 * k2]
                    else:
                        src = pHI[:, 128 + 32 * (k2 - 1):192 + 32 * (k2 - 1)]
                    nc.vector.tensor_tensor(
                        oA[:, 256 + 64 * k2:320 + 64 * k2],
                        src, scal[:, 64:128], op=A.mult)
                nc.sync.dma_start(out=outA[w * P:(w + 1) * P, :], in_=oA[:])

            # lag-1 software pipeline: next window's ph1 matmuls are
            # queued ahead of this window's serial ph2 chain.
            prev = None
            for w in range(W):
                cur = ph1(w)
                if prev is not None:
                    ph2(w - 1, *prev)
                prev = cur
            ph2(W - 1, *prev)

    nc.compile()
    return nc


MSG_DTYPE = os.environ.get("KERNEL_MSG_DTYPE", "bf16")


def kernel(msg0, msg1, msg2, index, num_nodes,
           W_s1, b_s1, W_s2, b_s2, W_L1, W_L2, W_g1, b_g1, W_g2, b_g2):
    global LAST_EXEC_NS, LAST_RESULTS
    from concourse import bass_utils

    if MSG_DTYPE == "bf16":
        import ml_dtypes
        msg_np = ml_dtypes.bfloat16
    else:
        msg_np = np.float32
    E = int(np.asarray(index).shape[0])
    N = int(np.asarray(num_nodes))

    idx = np.asarray(index).astype(np.int64).ravel()
    perm = np.argsort(idx, kind="stable")
    sidx = idx[perm]

    starts, ncnt, ecnt = _pack_windows(idx, N)
    Wt = len(starts)
    Wc = -(-Wt // NCORES)           # windows per core
    Wpad = Wc * NCORES

    # slot layout
    E0 = np.concatenate(([0], np.cumsum(ecnt)))[:-1]
    win_of_edge = np.repeat(np.arange(Wt), ecnt)
    slot = win_of_edge * CAP + (np.arange(E) - E0[win_of_edge])

    lidx_g = np.full(Wpad * CAP, -1.0, np.float32)
    lidx_g[slot] = (sidx - starts[win_of_edge]).astype(np.float32)

    msgs_g = np.zeros((Wpad * CAP, FT), msg_np)
    m0 = np.asarray(msg0, np.float32)
    m1 = np.asarray(msg1, np.float32).reshape(E, 192)
    m2 = np.asarray(msg2, np.float32).reshape(E, 576)
    msgs_g[slot, 0:64] = m0[perm]
    msgs_g[slot, 64:256] = m1[perm]
    msgs_g[slot, 256:832] = m2[perm]
    # repack so row (w, p) = [edge slots w*CAP + k*128 + p for k in 0..TPW)
    # -> one descriptor per partition per window
    msgs_g = np.ascontiguousarray(
        msgs_g.reshape(Wpad, TPW, P, FT).transpose(0, 2, 1, 3)
    ).reshape(Wpad * P, TPW * FT)

    # weights / constants
    W_s1 = np.asarray(W_s1, np.float32)
    fold = np.zeros((128, 64), np.float32)
    fold[np.arange(128), np.arange(128) % 64] = 1.0
    selhi = np.zeros((128, 64), np.float32)
    selhi[np.arange(64) + 64, np.arange(64)] = 1.0
    cst = {
        "iota": np.ascontiguousarray(
            np.broadcast_to(np.arange(P, dtype=np.float32), (P, P))
        ).astype(msg_np),
        "ws1t0": np.ascontiguousarray(W_s1.T[0:64]).astype(msg_np),
        "ws1t1": np.ascontiguousarray(W_s1.T[64:128]).astype(msg_np),
        "ws1t2": np.ascontiguousarray(W_s1.T[128:192]).astype(msg_np),
        "fold": fold.astype(msg_np),
        "selhi": selhi.astype(msg_np),
        "id64": np.eye(64, dtype=np.float32).astype(msg_np),
        "ws2": np.ascontiguousarray(
            np.asarray(W_s2, np.float32).T).astype(msg_np),
        "wg1": np.ascontiguousarray(
            np.asarray(W_g1, np.float32).T).astype(msg_np),
        "wg2": np.ascontiguousarray(
            np.asarray(W_g2, np.float32).T).astype(msg_np),
        "wl1": np.ascontiguousarray(
            np.vstack([np.asarray(W_L1, np.float32).T] * 2)).astype(msg_np),
        "wl2": np.ascontiguousarray(
            np.vstack([np.asarray(W_L2, np.float32).T] * 2)).astype(msg_np),
        "bs1": np.asarray(b_s1, np.float32).reshape(64, 1),
        "bg1": np.asarray(b_g1, np.float32).reshape(64, 1),
        "bs2c": np.asarray(b_s2, np.float32).reshape(64, 1),
        "bs2b": np.ascontiguousarray(
            np.broadcast_to(np.asarray(b_s2, np.float32), (P, 64))),
        "bg2b": np.ascontiguousarray(
            np.broadcast_to(np.asarray(b_g2, np.float32), (P, 128))),
        "eps": np.full((64, 1), 1e-8, np.float32),
    }

    nc = _build_program(Wc, msg_np)

    in_maps = []
    for c in range(NCORES):
        lo, hi = c * Wc * CAP, (c + 1) * Wc * CAP
        lidx_c = np.ascontiguousarray(
            lidx_g[lo:hi].reshape(Wc * TPW, P).T)
        in_maps.append({"msgs": msgs_g[c * Wc * P:(c + 1) * Wc * P],
                        "lidx": lidx_c, **cst})

    trace = os.environ.get("KERNEL_PROFILE", "0") == "1"
    if trace:
        _install_ntff_hook()
    res = bass_utils.run_bass_kernel_spmd(
        nc, in_maps, core_ids=list(range(NCORES)), trace=trace)
    LAST_RESULTS = res
    LAST_EXEC_NS = res.exec_time_ns

    # unpack outputs
    delta0 = np.empty((N, 64), np.float32)
    delta1 = np.empty((N, 192), np.float32)
    delta2 = np.empty((N, 576), np.float32)
    win_of_node = np.repeat(np.arange(Wt), ncnt)
    pos = np.arange(N) - starts[win_of_node]
    rows = (win_of_node % Wc) * P + pos
    cores = win_of_node // Wc
    for c in range(NCORES):
        m = cores == c
        if not m.any():
            continue
        r = rows[m]
        oa = np.asarray(res.results[c]["outA"])
        delta0[m] = oa[r, 0:64]
        delta1[m] = oa[r, 64:256]
        delta2[m] = oa[r, 256:832]

    return (delta0, delta1.reshape(N, 3, 64), delta2.reshape(N, 3, 3, 64))


# revision 61
# speedup vs baseline: 1.0132x; 1.0132x over previous
"""Trainium2 Bass kernel for CartesianDensityBlock (GNN message passing).

Strategy:
  * Host: sort edges by destination node; greedily pack consecutive nodes
    into "windows" of <=128 nodes and <=640 edges (5 tiles of 128 edge
    slots).  Windows are distributed contiguously across 8 cores, so every
    node's edges live on exactly one core -> no collectives.
  * Device (per window): segment-sum via one-hot matmuls on TensorE
    producing feature-major densities denT [832f, 128n] in PSUM, then
    rotation invariants + MLPs + channel-mix + gating entirely on-chip,
    emitting node-major outputs.
  * Host: scatter per-window rows back to the full [N, ...] outputs.
"""

import os
import sys

import numpy as np

for _p in ("/opt/trn_rl_repo",):
    if _p not in sys.path:
        sys.path.insert(0, _p)

P = 128
TPW = 5                # edge tiles per window
CAP = TPW * P          # max edges per window
NCORES = 8
FT = 832               # 64 + 3*64 + 9*64 features per edge
INV_SQRT_DEG = 1.0 / 50.0 ** 0.5

# set KERNEL_PROFILE=1 in the environment to capture an NTFF profile
LAST_EXEC_NS = None
LAST_RESULTS = None

_AXON_SO = "/opt/axon/libaxon_pjrt.so"


def _install_ntff_hook():
    """Provide antenv.axon_hooks (absent in this image) so that
    run_bass_kernel_spmd(trace=True) can capture NTFF profiles."""
    import types
    import ctypes
    import contextlib

    try:
        from antenv.axon_hooks import get_axon_ntff_profile_hook  # noqa
        return
    except ImportError:
        pass
    if not os.path.exists(_AXON_SO):
        return

    lib = ctypes.CDLL(_AXON_SO)
    if not hasattr(lib, "axon_start_nrt_profile"):
        return
    lib.axon_start_nrt_profile.argtypes = [
        ctypes.POINTER(ctypes.c_int64), ctypes.c_size_t]
    lib.axon_start_nrt_profile.restype = ctypes.c_int64
    lib.axon_stop_nrt_profile.argtypes = [ctypes.c_char_p]
    lib.axon_stop_nrt_profile.restype = ctypes.c_int64

    @contextlib.contextmanager
    def _hook(output_dir, device_ids):
        import jax
        jax.devices()
        if device_ids:
            ids = (ctypes.c_int64 * len(device_ids))(*device_ids)
            rc = lib.axon_start_nrt_profile(ids, len(device_ids))
        else:
            rc = lib.axon_start_nrt_profile(None, 0)
        if rc != 0:
            raise RuntimeError(f"axon_start_nrt_profile rc={rc}")
        try:
            yield
        finally:
            n = lib.axon_stop_nrt_profile(str(output_dir).encode())
            print(f"profile: {n} file(s) written to {output_dir}",
                  file=sys.stderr)

    mod = types.ModuleType("antenv.axon_hooks")
    mod._hook = _hook
    mod.get_axon_ntff_profile_hook = lambda: _hook
    mod.set_axon_ntff_profile_hook = lambda h: None
    import antenv
    antenv.axon_hooks = mod
    sys.modules["antenv.axon_hooks"] = mod


def _pack_windows(idx, num_nodes):
    """Greedy packing of consecutive (sorted) nodes into windows."""
    counts = np.bincount(idx, minlength=num_nodes)
    assert counts.max() <= CAP, "node degree exceeds window capacity"
    starts, ncnt, ecnt = [], [], []
    n0 = 0
    while n0 < num_nodes:
        hi = min(n0 + P, num_nodes)
        c = np.cumsum(counts[n0:hi])
        k = int(np.searchsorted(c, CAP, side="right"))
        k = max(k, 1)
        starts.append(n0)
        ncnt.append(k)
        ecnt.append(int(c[k - 1]))
        n0 += k
    return (np.asarray(starts, np.int64), np.asarray(ncnt, np.int64),
            np.asarray(ecnt, np.int64))


def _build_program(W, msg_dt_np, stage=99):
    import concourse.bacc as bacc
    import concourse.mybir as mybir
    import concourse.tile as tile

    dt = mybir.dt
    f32 = dt.float32
    mdt = dt.from_np(np.dtype(msg_dt_np))
    A = mybir.AluOpType
    AF = mybir.ActivationFunctionType

    nc = bacc.Bacc("TRN2", target_bir_lowering=False, debug=False)

    # msgs layout: [W*128 rows, TPW*FT] — row (w, p) holds the feature
    # vectors of the 5 edges that land on partition p in window w, so a
    # whole window loads as ONE DMA with one descriptor per partition.
    msgs = nc.dram_tensor("msgs", [W * P, TPW * FT], mdt,
                          kind="ExternalInput")
    lidx = nc.dram_tensor("lidx", [P, W * TPW], f32, kind="ExternalInput")
    iota = nc.dram_tensor("iota", [P, P], mdt, kind="ExternalInput")
    ws1t0 = nc.dram_tensor("ws1t0", [64, 64], mdt, kind="ExternalInput")
    ws1t1 = nc.dram_tensor("ws1t1", [64, 64], mdt, kind="ExternalInput")
    ws1t2 = nc.dram_tensor("ws1t2", [64, 64], mdt, kind="ExternalInput")
    fold = nc.dram_tensor("fold", [128, 64], mdt, kind="ExternalInput")
    selhi = nc.dram_tensor("selhi", [128, 64], mdt, kind="ExternalInput")
    id64 = nc.dram_tensor("id64", [64, 64], mdt, kind="ExternalInput")
    ws2 = nc.dram_tensor("ws2", [64, 64], mdt, kind="ExternalInput")
    wg1 = nc.dram_tensor("wg1", [64, 64], mdt, kind="ExternalInput")
    wg2 = nc.dram_tensor("wg2", [64, 128], mdt, kind="ExternalInput")
    wl1 = nc.dram_tensor("wl1", [128, 64], mdt, kind="ExternalInput")
    wl2 = nc.dram_tensor("wl2", [128, 64], mdt, kind="ExternalInput")
    bs1 = nc.dram_tensor("bs1", [64, 1], f32, kind="ExternalInput")
    bg1 = nc.dram_tensor("bg1", [64, 1], f32, kind="ExternalInput")
    bs2c = nc.dram_tensor("bs2c", [64, 1], f32, kind="ExternalInput")
    bs2b = nc.dram_tensor("bs2b", [P, 64], f32, kind="ExternalInput")
    bg2b = nc.dram_tensor("bg2b", [P, 128], f32, kind="ExternalInput")
    eps = nc.dram_tensor("eps", [64, 1], f32, kind="ExternalInput")

    # single merged output: [delta_h0 (64) | delta_h1 (192) | delta_h2
    # (576)] per node row -> one DMA per window.
    outA = nc.dram_tensor("outA", [W * P, FT], f32, kind="ExternalOutput")

    with tile.TileContext(nc) as tc:
        with (
            tc.tile_pool(name="const", bufs=1) as cp,
            tc.tile_pool(name="mpool", bufs=6) as mp,
            tc.tile_pool(name="ohpool", bufs=20) as ohp,
            tc.tile_pool(name="work", bufs=3) as wp,
            tc.tile_pool(name="outp", bufs=4) as op,
            tc.tile_pool(name="pden", bufs=2, space="PSUM") as pden,
            tc.tile_pool(name="pmlp", bufs=4, space="PSUM") as pmlp,
        ):
            def cload(dram, shape, dtype=f32):
                t = cp.tile(shape, dtype, tag=dram.name)
                nc.sync.dma_start(out=t[:], in_=dram[:])
                return t

            iota_t = cload(iota, [P, P], mdt)
            lidx_t = cload(lidx, [P, W * TPW])
            ws1t0_t = cload(ws1t0, [64, 64], mdt)
            ws1t1_t = cload(ws1t1, [64, 64], mdt)
            ws1t2_t = cload(ws1t2, [64, 64], mdt)
            fold_t = cload(fold, [128, 64], mdt)
            selhi_t = cload(selhi, [128, 64], mdt)
            id64_t = cload(id64, [64, 64], mdt)
            ws2_t = cload(ws2, [64, 64], mdt)
            wg1_t = cload(wg1, [64, 64], mdt)
            wg2_t = cload(wg2, [64, 128], mdt)
            wl1_t = cload(wl1, [128, 64], mdt)
            wl2_t = cload(wl2, [128, 64], mdt)
            bs1_t = cload(bs1, [64, 1])
            bg1_t = cload(bg1, [64, 1])
            bs2c_t = cload(bs2c, [64, 1])
            bs2b_t = cload(bs2b, [P, 64])
            bg2b_t = cload(bg2b, [P, 128])
            eps_t = cload(eps, [64, 1])

            def ph1(w):
                # ---------- phase 1: segment-sum into denT (PSUM) ----------
                # chunk-major matmul order: exactly one open accumulation
                # group per PSUM bank at any time.
                pA = pden.tile([P, 512], f32, tag="pA", name=f"pA{w}")
                pB = pden.tile([P, 384], f32, tag="pB", name=f"pB{w}")
                mt = mp.tile([P, TPW * FT], mdt, tag="mt", name=f"mt{w}")
                nc.sync.dma_start(out=mt[:], in_=msgs[w * P:(w + 1) * P, :])
                ohs = []
                for k in range(TPW):
                    g = w * TPW + k
                    oh = ohp.tile([P, P], mdt, tag="oh", name=f"oh{g}")
                    nc.vector.tensor_scalar(
                        oh[:], iota_t[:], lidx_t[:, g:g + 1], None, A.is_equal)
                    ohs.append(oh)
                for c in range(7):
                    lo = c * 128
                    hi = min(lo + 128, FT)
                    m = hi - lo
                    if c < 4:
                        dst = pA[:m, lo:lo + 128]
                    else:
                        dst = pB[:m, (c - 4) * 128:(c - 4) * 128 + 128]
                    for k in range(TPW):
                        nc.tensor.matmul(dst,
                                         lhsT=mt[:, k * FT + lo:k * FT + hi],
                                         rhs=ohs[k][:],
                                         start=(k == 0), stop=(k == TPW - 1))
                return pA, pB

            def ph2a(w, pA, pB):
                """Scale/square copies, invariant folds, channel-mix
                matmuls — everything whose deps clear quickly."""
                if stage < 1:
                    return None
                s = INV_SQRT_DEG
                sA = wp.tile([P, 512], mdt, tag="sA", name=f"sA{w}")
                nc.vector.tensor_scalar_mul(sA[:], pA[:], s)
                sB1 = wp.tile([P, 256], mdt, tag="sB1", name=f"sB1_{w}")
                nc.vector.tensor_scalar_mul(sB1[:], pB[:, 0:256], s)
                sB2 = wp.tile([64, 128], mdt, tag="sB2", name=f"sB2_{w}")
                nc.vector.tensor_scalar_mul(sB2[:], pB[0:64, 256:384], s)

                sqA = wp.tile([P, 512], mdt, tag="sqA", name=f"sqA{w}")
                nc.vector.tensor_tensor(sqA[:], sA[:], sA[:], op=A.mult)
                sqB1 = wp.tile([P, 256], mdt, tag="sqB1", name=f"sqB1_{w}")
                nc.vector.tensor_tensor(sqB1[:], sB1[:], sB1[:], op=A.mult)
                sqB2 = wp.tile([64, 128], mdt, tag="sqB2", name=f"sqB2_{w}")
                nc.vector.tensor_tensor(sqB2[:], sB2[:], sB2[:], op=A.mult)

                # channel-mix matmuls; base-0 vs base-64 sourced groups in
                # separate PSUM banks (disjoint PE row-groups run
                # concurrently and must not share a bank)
                d1s = (sA[64:128, 0:128], sA[0:64, 128:256],
                       sA[64:128, 128:256])
                d2s = (sA[0:64, 256:384], sA[64:128, 256:384],
                       sA[0:64, 384:512], sA[64:128, 384:512],
                       sB1[0:64, 0:128], sB1[64:128, 0:128],
                       sB1[0:64, 128:256], sB1[64:128, 128:256],
                       sB2[:, :])
                pLO = pmlp.tile([P, 384], f32, tag="pm", name=f"pLO{w}")
                pHI = pmlp.tile([P, 384], f32, tag="pm", name=f"pHI{w}")
                los = (d1s[1], d2s[0], d2s[2], d2s[4], d2s[6], d2s[8])
                his = (d1s[0], d1s[2], d2s[1], d2s[3], d2s[5], d2s[7])
                for j, dsrc in enumerate(los):
                    wmix = wl2_t if j else wl1_t
                    nc.tensor.matmul(pLO[:, 64 * j:64 * j + 64],
                                     lhsT=dsrc, rhs=wmix[0:64, :],
                                     start=True, stop=True)
                for j, dsrc in enumerate(his):
                    wmix = wl2_t if j >= 2 else wl1_t
                    nc.tensor.matmul(pHI[:, 64 * j:64 * j + 64],
                                     lhsT=dsrc, rhs=wmix[64:128, :],
                                     start=True, stop=True)
                sLO = wp.tile([P, 384], f32, tag="sLO", name=f"sLO{w}")
                nc.vector.tensor_copy(out=sLO[:], in_=pLO[:])
                sHI = wp.tile([P, 384], f32, tag="sHI", name=f"sHI{w}")
                nc.vector.tensor_copy(out=sHI[:], in_=pHI[:])

                # invariant folds on TensorE with 0/1 selection matrices
                pi1 = pmlp.tile([64, 128], f32, tag="pm", name=f"pi1_{w}")
                nc.tensor.matmul(pi1[:], lhsT=selhi_t[:], rhs=sqA[:, 0:128],
                                 start=True, stop=False)
                nc.tensor.matmul(pi1[:], lhsT=fold_t[:], rhs=sqA[:, 128:256],
                                 start=False, stop=True)
                v1 = wp.tile([64, 128], mdt, tag="v1", name=f"v1_{w}")
                nc.scalar.activation(v1[:], pi1[:], AF.Sqrt,
                                     bias=eps_t[:, 0:1])
                pi2 = pmlp.tile([64, 128], f32, tag="pm", name=f"pi2_{w}")
                nc.tensor.matmul(pi2[:], lhsT=fold_t[:], rhs=sqA[:, 256:384],
                                 start=True, stop=False)
                nc.tensor.matmul(pi2[:], lhsT=fold_t[:], rhs=sqA[:, 384:512],
                                 start=False, stop=False)
                nc.tensor.matmul(pi2[:], lhsT=fold_t[:], rhs=sqB1[:, 0:128],
                                 start=False, stop=False)
                nc.tensor.matmul(pi2[:], lhsT=fold_t[:], rhs=sqB1[:, 128:256],
                                 start=False, stop=False)
                nc.tensor.matmul(pi2[:], lhsT=id64_t[:], rhs=sqB2[:],
                                 start=False, stop=True)
                v2 = wp.tile([64, 128], mdt, tag="v2", name=f"v2_{w}")
                nc.scalar.activation(v2[:], pi2[:], AF.Sqrt,
                                     bias=eps_t[:, 0:1])
                return dict(sA=sA, v1=v1, v2=v2, sLO=sLO, sHI=sHI)

            def ph2b(w, ctx):
                """Serial scalar-update MLP + gating chain, one window
                behind ph2a so its cross-engine latencies are hidden."""
                if ctx is None:
                    return
                sA, v1, v2 = ctx["sA"], ctx["v1"], ctx["v2"]
                sLO, sHI = ctx["sLO"], ctx["sHI"]
                p1 = pmlp.tile([64, 128], f32, tag="pm", name=f"p1_{w}")
                nc.tensor.matmul(p1[:], lhsT=ws1t0_t[:], rhs=sA[0:64, 0:128],
                                 start=True, stop=False)
                nc.tensor.matmul(p1[:], lhsT=ws1t1_t[:], rhs=v1[:],
                                 start=False, stop=False)
                nc.tensor.matmul(p1[:], lhsT=ws1t2_t[:], rhs=v2[:],
                                 start=False, stop=True)
                hx = wp.tile([64, 128], mdt, tag="hx", name=f"hx{w}")
                nc.vector.tensor_scalar(hx[:], p1[:], bs1_t[:, 0:1], None,
                                        A.add)
                hs = wp.tile([64, 128], mdt, tag="hs", name=f"hs{w}")
                nc.scalar.activation(hs[:], p1[:], AF.Sigmoid,
                                     bias=bs1_t[:, 0:1])
                hT = wp.tile([64, 128], mdt, tag="hT", name=f"hT{w}")
                nc.vector.tensor_tensor(hT[:], hx[:], hs[:], op=A.mult)

                pd = pmlp.tile([64, 128], f32, tag="pm", name=f"pd{w}")
                nc.tensor.matmul(pd[:], lhsT=ws2_t[:], rhs=hT[:],
                                 start=True, stop=True)
                dh0T = wp.tile([64, 128], mdt, tag="dh0T", name=f"dh0T{w}")
                nc.vector.tensor_scalar(dh0T[:], pd[:], bs2c_t[:, 0:1], None,
                                        A.add)

                pn0 = pmlp.tile([P, 64], f32, tag="pm", name=f"pn0_{w}")
                nc.tensor.matmul(pn0[:], lhsT=hT[:], rhs=ws2_t[:],
                                 start=True, stop=True)
                oA = op.tile([P, FT], f32, tag="oA", name=f"oA{w}")
                nc.vector.tensor_tensor(oA[:, 0:64], pn0[:], bs2b_t[:],
                                        op=A.add)

                pg = pmlp.tile([64, 128], f32, tag="pm", name=f"pg{w}")
                nc.tensor.matmul(pg[:], lhsT=wg1_t[:], rhs=dh0T[:],
                                 start=True, stop=True)
                gx = wp.tile([64, 128], mdt, tag="gx", name=f"gx{w}")
                nc.vector.tensor_scalar(gx[:], pg[:], bg1_t[:, 0:1], None,
                                        A.add)
                gs = wp.tile([64, 128], mdt, tag="gs", name=f"gs{w}")
                nc.scalar.activation(gs[:], pg[:], AF.Sigmoid,
                                     bias=bg1_t[:, 0:1])
                hgT = wp.tile([64, 128], mdt, tag="hgT", name=f"hgT{w}")
                nc.vector.tensor_tensor(hgT[:], gx[:], gs[:], op=A.mult)

                ps = pmlp.tile([P, 128], f32, tag="pm", name=f"ps{w}")
                nc.tensor.matmul(ps[:], lhsT=hgT[:], rhs=wg2_t[:],
                                 start=True, stop=True)
                scal = wp.tile([P, 128], f32, tag="scal", name=f"scal{w}")
                nc.vector.tensor_tensor(scal[:], ps[:], bg2b_t[:], op=A.add)

                # gating
                d1src = (sHI[:, 0:64], sLO[:, 0:64], sHI[:, 64:128])
                for i in range(3):
                    nc.vector.tensor_tensor(oA[:, 64 + 64 * i:128 + 64 * i],
                                            d1src[i], scal[:, 0:64],
                                            op=A.mult)
                for k2 in range(9):
                    if k2 % 2 == 0:
                        src = sLO[:, 64 + 32 * k2:128 + 32 * k2]
                    else:
                        src = sHI[:, 128 + 32 * (k2 - 1):192 + 32 * (k2 - 1)]
                    nc.vector.tensor_tensor(
                        oA[:, 256 + 64 * k2:320 + 64 * k2],
                        src, scal[:, 64:128], op=A.mult)
                nc.sync.dma_start(out=outA[w * P:(w + 1) * P, :], in_=oA[:])

            # staggered pipeline: ph1(w) | ph2a(w-1) | ph2b(w-2)
            dens, ctxs = {}, {}
            for w in range(W):
                dens[w] = ph1(w)
                if w >= 1:
                    ctxs[w - 1] = ph2a(w - 1, *dens.pop(w - 1))
                if w >= 2:
                    ph2b(w - 2, ctxs.pop(w - 2))
            ctxs[W - 1] = ph2a(W - 1, *dens.pop(W - 1))
            for w in sorted(ctxs):
                ph2b(w, ctxs.pop(w))

    nc.compile()
    return nc


MSG_DTYPE = os.environ.get("KERNEL_MSG_DTYPE", "bf16")


def kernel(msg0, msg1, msg2, index, num_nodes,
           W_s1, b_s1, W_s2, b_s2, W_L1, W_L2, W_g1, b_g1, W_g2, b_g2):
    global LAST_EXEC_NS, LAST_RESULTS
    from concourse import bass_utils

    if MSG_DTYPE == "bf16":
        import ml_dtypes
        msg_np = ml_dtypes.bfloat16
    else:
        msg_np = np.float32
    E = int(np.asarray(index).shape[0])
    N = int(np.asarray(num_nodes))

    idx = np.asarray(index).astype(np.int64).ravel()
    perm = np.argsort(idx, kind="stable")
    sidx = idx[perm]

    starts, ncnt, ecnt = _pack_windows(idx, N)
    Wt = len(starts)
    Wc = -(-Wt // NCORES)           # windows per core
    Wpad = Wc * NCORES

    # slot layout
    E0 = np.concatenate(([0], np.cumsum(ecnt)))[:-1]
    win_of_edge = np.repeat(np.arange(Wt), ecnt)
    slot = win_of_edge * CAP + (np.arange(E) - E0[win_of_edge])

    lidx_g = np.full(Wpad * CAP, -1.0, np.float32)
    lidx_g[slot] = (sidx - starts[win_of_edge]).astype(np.float32)

    msgs_g = np.zeros((Wpad * CAP, FT), msg_np)
    m0 = np.asarray(msg0, np.float32)
    m1 = np.asarray(msg1, np.float32).reshape(E, 192)
    m2 = np.asarray(msg2, np.float32).reshape(E, 576)
    msgs_g[slot, 0:64] = m0[perm]
    msgs_g[slot, 64:256] = m1[perm]
    msgs_g[slot, 256:832] = m2[perm]
    # repack so row (w, p) = [edge slots w*CAP + k*128 + p for k in 0..TPW)
    # -> one descriptor per partition per window
    msgs_g = np.ascontiguousarray(
        msgs_g.reshape(Wpad, TPW, P, FT).transpose(0, 2, 1, 3)
    ).reshape(Wpad * P, TPW * FT)

    # weights / constants
    W_s1 = np.asarray(W_s1, np.float32)
    fold = np.zeros((128, 64), np.float32)
    fold[np.arange(128), np.arange(128) % 64] = 1.0
    selhi = np.zeros((128, 64), np.float32)
    selhi[np.arange(64) + 64, np.arange(64)] = 1.0
    cst = {
        "iota": np.ascontiguousarray(
            np.broadcast_to(np.arange(P, dtype=np.float32), (P, P))
        ).astype(msg_np),
        "ws1t0": np.ascontiguousarray(W_s1.T[0:64]).astype(msg_np),
        "ws1t1": np.ascontiguousarray(W_s1.T[64:128]).astype(msg_np),
        "ws1t2": np.ascontiguousarray(W_s1.T[128:192]).astype(msg_np),
        "fold": fold.astype(msg_np),
        "selhi": selhi.astype(msg_np),
        "id64": np.eye(64, dtype=np.float32).astype(msg_np),
        "ws2": np.ascontiguousarray(
            np.asarray(W_s2, np.float32).T).astype(msg_np),
        "wg1": np.ascontiguousarray(
            np.asarray(W_g1, np.float32).T).astype(msg_np),
        "wg2": np.ascontiguousarray(
            np.asarray(W_g2, np.float32).T).astype(msg_np),
        "wl1": np.ascontiguousarray(
            np.vstack([np.asarray(W_L1, np.float32).T] * 2)).astype(msg_np),
        "wl2": np.ascontiguousarray(
            np.vstack([np.asarray(W_L2, np.float32).T] * 2)).astype(msg_np),
        "bs1": np.asarray(b_s1, np.float32).reshape(64, 1),
        "bg1": np.asarray(b_g1, np.float32).reshape(64, 1),
        "bs2c": np.asarray(b_s2, np.float32).reshape(64, 1),
        "bs2b": np.ascontiguousarray(
            np.broadcast_to(np.asarray(b_s2, np.float32), (P, 64))),
        "bg2b": np.ascontiguousarray(
            np.broadcast_to(np.asarray(b_g2, np.float32), (P, 128))),
        "eps": np.full((64, 1), 1e-8, np.float32),
    }

    nc = _build_program(Wc, msg_np)

    in_maps = []
    for c in range(NCORES):
        lo, hi = c * Wc * CAP, (c + 1) * Wc * CAP
        lidx_c = np.ascontiguousarray(
            lidx_g[lo:hi].reshape(Wc * TPW, P).T)
        in_maps.append({"msgs": msgs_g[c * Wc * P:(c + 1) * Wc * P],
                        "lidx": lidx_c, **cst})

    trace = os.environ.get("KERNEL_PROFILE", "0") == "1"
    if trace:
        _install_ntff_hook()
    res = bass_utils.run_bass_kernel_spmd(
        nc, in_maps, core_ids=list(range(NCORES)), trace=trace)
    LAST_RESULTS = res
    LAST_EXEC_NS = res.exec_time_ns

    # unpack outputs
    delta0 = np.empty((N, 64), np.float32)
    delta1 = np.empty((N, 192), np.float32)
    delta2 = np.empty((N, 576), np.float32)
    win_of_node = np.repeat(np.arange(Wt), ncnt)
    pos = np.arange(N) - starts[win_of_node]
    rows = (win_of_node % Wc) * P + pos
    cores = win_of_node // Wc
    for c in range(NCORES):
        m = cores == c
        if not m.any():
            continue
        r = rows[m]
        oa = np.asarray(res.results[c]["outA"])
        delta0[m] = oa[r, 0:64]
        delta1[m] = oa[r, 64:256]
        delta2[m] = oa[r, 256:832]

    return (delta0, delta1.reshape(N, 3, 64), delta2.reshape(N, 3, 3, 64))


# revision 62
# speedup vs baseline: 1.1162x; 1.1016x over previous
"""Trainium2 Bass kernel for CartesianDensityBlock (GNN message passing).

Strategy:
  * Host: sort edges by destination node; greedily pack consecutive nodes
    into "windows" of <=128 nodes and <=640 edges (5 tiles of 128 edge
    slots).  Windows are distributed contiguously across 8 cores, so every
    node's edges live on exactly one core -> no collectives.
  * Device (per window): segment-sum via one-hot matmuls on TensorE
    producing feature-major densities denT [832f, 128n] in PSUM, then
    rotation invariants + MLPs + channel-mix + gating entirely on-chip,
    emitting node-major outputs.
  * Host: scatter per-window rows back to the full [N, ...] outputs.
"""

import os
import sys

import numpy as np

for _p in ("/opt/trn_rl_repo",):
    if _p not in sys.path:
        sys.path.insert(0, _p)

P = 128
TPW = 5                # edge tiles per window
CAP = TPW * P          # max edges per window
NCORES = 8
FT = 832               # 64 + 3*64 + 9*64 features per edge
INV_SQRT_DEG = 1.0 / 50.0 ** 0.5

# set KERNEL_PROFILE=1 in the environment to capture an NTFF profile
LAST_EXEC_NS = None
LAST_RESULTS = None

_AXON_SO = "/opt/axon/libaxon_pjrt.so"


def _install_ntff_hook():
    """Provide antenv.axon_hooks (absent in this image) so that
    run_bass_kernel_spmd(trace=True) can capture NTFF profiles."""
    import types
    import ctypes
    import contextlib

    try:
        from antenv.axon_hooks import get_axon_ntff_profile_hook  # noqa
        return
    except ImportError:
        pass
    if not os.path.exists(_AXON_SO):
        return

    lib = ctypes.CDLL(_AXON_SO)
    if not hasattr(lib, "axon_start_nrt_profile"):
        return
    lib.axon_start_nrt_profile.argtypes = [
        ctypes.POINTER(ctypes.c_int64), ctypes.c_size_t]
    lib.axon_start_nrt_profile.restype = ctypes.c_int64
    lib.axon_stop_nrt_profile.argtypes = [ctypes.c_char_p]
    lib.axon_stop_nrt_profile.restype = ctypes.c_int64

    @contextlib.contextmanager
    def _hook(output_dir, device_ids):
        import jax
        jax.devices()
        if device_ids:
            ids = (ctypes.c_int64 * len(device_ids))(*device_ids)
            rc = lib.axon_start_nrt_profile(ids, len(device_ids))
        else:
            rc = lib.axon_start_nrt_profile(None, 0)
        if rc != 0:
            raise RuntimeError(f"axon_start_nrt_profile rc={rc}")
        try:
            yield
        finally:
            n = lib.axon_stop_nrt_profile(str(output_dir).encode())
            print(f"profile: {n} file(s) written to {output_dir}",
                  file=sys.stderr)

    mod = types.ModuleType("antenv.axon_hooks")
    mod._hook = _hook
    mod.get_axon_ntff_profile_hook = lambda: _hook
    mod.set_axon_ntff_profile_hook = lambda h: None
    import antenv
    antenv.axon_hooks = mod
    sys.modules["antenv.axon_hooks"] = mod


def _pack_windows(idx, num_nodes):
    """Greedy packing of consecutive (sorted) nodes into windows."""
    counts = np.bincount(idx, minlength=num_nodes)
    assert counts.max() <= CAP, "node degree exceeds window capacity"
    starts, ncnt, ecnt = [], [], []
    n0 = 0
    while n0 < num_nodes:
        hi = min(n0 + P, num_nodes)
        c = np.cumsum(counts[n0:hi])
        k = int(np.searchsorted(c, CAP, side="right"))
        k = max(k, 1)
        starts.append(n0)
        ncnt.append(k)
        ecnt.append(int(c[k - 1]))
        n0 += k
    return (np.asarray(starts, np.int64), np.asarray(ncnt, np.int64),
            np.asarray(ecnt, np.int64))


def _build_program(W, msg_dt_np, stage=99):
    import concourse.bacc as bacc
    import concourse.mybir as mybir
    import concourse.tile as tile

    dt = mybir.dt
    f32 = dt.float32
    mdt = dt.from_np(np.dtype(msg_dt_np))
    A = mybir.AluOpType
    AF = mybir.ActivationFunctionType

    nc = bacc.Bacc("TRN2", target_bir_lowering=False, debug=False)

    # msgs layout: [W*128 rows, TPW*FT] — row (w, p) holds the feature
    # vectors of the 5 edges that land on partition p in window w, so a
    # whole window loads as ONE DMA with one descriptor per partition.
    msgs = nc.dram_tensor("msgs", [W * P, TPW * FT], mdt,
                          kind="ExternalInput")
    lidx = nc.dram_tensor("lidx", [P, W * TPW], f32, kind="ExternalInput")
    iota = nc.dram_tensor("iota", [P, P], mdt, kind="ExternalInput")
    ws1t0 = nc.dram_tensor("ws1t0", [64, 64], mdt, kind="ExternalInput")
    ws1t1 = nc.dram_tensor("ws1t1", [64, 64], mdt, kind="ExternalInput")
    ws1t2 = nc.dram_tensor("ws1t2", [64, 64], mdt, kind="ExternalInput")
    fold = nc.dram_tensor("fold", [128, 64], mdt, kind="ExternalInput")
    selhi = nc.dram_tensor("selhi", [128, 64], mdt, kind="ExternalInput")
    id64 = nc.dram_tensor("id64", [64, 64], mdt, kind="ExternalInput")
    ws2 = nc.dram_tensor("ws2", [64, 64], mdt, kind="ExternalInput")
    wg1 = nc.dram_tensor("wg1", [64, 64], mdt, kind="ExternalInput")
    wg2 = nc.dram_tensor("wg2", [64, 128], mdt, kind="ExternalInput")
    wl1 = nc.dram_tensor("wl1", [128, 64], mdt, kind="ExternalInput")
    wl2 = nc.dram_tensor("wl2", [128, 64], mdt, kind="ExternalInput")
    bs1 = nc.dram_tensor("bs1", [64, 1], f32, kind="ExternalInput")
    bg1 = nc.dram_tensor("bg1", [64, 1], f32, kind="ExternalInput")
    bs2c = nc.dram_tensor("bs2c", [64, 1], f32, kind="ExternalInput")
    bs2b = nc.dram_tensor("bs2b", [P, 64], f32, kind="ExternalInput")
    bg2b = nc.dram_tensor("bg2b", [P, 128], f32, kind="ExternalInput")
    eps = nc.dram_tensor("eps", [64, 1], f32, kind="ExternalInput")

    # single merged output: [delta_h0 (64) | delta_h1 (192) | delta_h2
    # (576)] per node row -> one DMA per window.
    outA = nc.dram_tensor("outA", [W * P, FT], f32, kind="ExternalOutput")

    with tile.TileContext(nc) as tc:
        with (
            tc.tile_pool(name="const", bufs=1) as cp,
            tc.tile_pool(name="mpool", bufs=6) as mp,
            tc.tile_pool(name="ohpool", bufs=20) as ohp,
            tc.tile_pool(name="work", bufs=3) as wp,
            tc.tile_pool(name="outp", bufs=4) as op,
            tc.tile_pool(name="pden", bufs=2, space="PSUM") as pden,
            tc.tile_pool(name="pmlp", bufs=4, space="PSUM") as pmlp,
        ):
            def cload(dram, shape, dtype=f32):
                t = cp.tile(shape, dtype, tag=dram.name)
                nc.sync.dma_start(out=t[:], in_=dram[:])
                return t

            iota_t = cload(iota, [P, P], mdt)
            lidx_t = cload(lidx, [P, W * TPW])
            ws1t0_t = cload(ws1t0, [64, 64], mdt)
            ws1t1_t = cload(ws1t1, [64, 64], mdt)
            ws1t2_t = cload(ws1t2, [64, 64], mdt)
            fold_t = cload(fold, [128, 64], mdt)
            selhi_t = cload(selhi, [128, 64], mdt)
            id64_t = cload(id64, [64, 64], mdt)
            ws2_t = cload(ws2, [64, 64], mdt)
            wg1_t = cload(wg1, [64, 64], mdt)
            wg2_t = cload(wg2, [64, 128], mdt)
            wl1_t = cload(wl1, [128, 64], mdt)
            wl2_t = cload(wl2, [128, 64], mdt)
            bs1_t = cload(bs1, [64, 1])
            bg1_t = cload(bg1, [64, 1])
            bs2c_t = cload(bs2c, [64, 1])
            bs2b_t = cload(bs2b, [P, 64])
            bg2b_t = cload(bg2b, [P, 128])
            eps_t = cload(eps, [64, 1])

            def ohbuild(w):
                ohs = []
                for k in range(TPW):
                    g = w * TPW + k
                    oh = ohp.tile([P, P], mdt, tag="oh", name=f"oh{g}")
                    nc.vector.tensor_scalar(
                        oh[:], iota_t[:], lidx_t[:, g:g + 1], None, A.is_equal)
                    ohs.append(oh)
                return ohs

            def ph1(w, ohs):
                # ---------- phase 1: segment-sum into denT (PSUM) ----------
                # chunk-major matmul order: exactly one open accumulation
                # group per PSUM bank at any time.
                pA = pden.tile([P, 512], f32, tag="pA", name=f"pA{w}")
                pB = pden.tile([P, 384], f32, tag="pB", name=f"pB{w}")
                mt = mp.tile([P, TPW * FT], mdt, tag="mt", name=f"mt{w}")
                nc.sync.dma_start(out=mt[:], in_=msgs[w * P:(w + 1) * P, :])
                for c in range(7):
                    lo = c * 128
                    hi = min(lo + 128, FT)
                    m = hi - lo
                    if c < 4:
                        dst = pA[:m, lo:lo + 128]
                    else:
                        dst = pB[:m, (c - 4) * 128:(c - 4) * 128 + 128]
                    for k in range(TPW):
                        nc.tensor.matmul(dst,
                                         lhsT=mt[:, k * FT + lo:k * FT + hi],
                                         rhs=ohs[k][:],
                                         start=(k == 0), stop=(k == TPW - 1))
                return pA, pB

            def ph2a(w, pA, pB):
                """Scale/square copies, invariant folds, channel-mix
                matmuls — everything whose deps clear quickly."""
                if stage < 1:
                    return None
                s = INV_SQRT_DEG
                sA = wp.tile([P, 512], mdt, tag="sA", name=f"sA{w}")
                nc.vector.tensor_scalar_mul(sA[:], pA[:], s)
                sB1 = wp.tile([P, 256], mdt, tag="sB1", name=f"sB1_{w}")
                nc.vector.tensor_scalar_mul(sB1[:], pB[:, 0:256], s)
                sB2 = wp.tile([64, 128], mdt, tag="sB2", name=f"sB2_{w}")
                nc.vector.tensor_scalar_mul(sB2[:], pB[0:64, 256:384], s)

                sqA = wp.tile([P, 512], mdt, tag="sqA", name=f"sqA{w}")
                nc.vector.tensor_tensor(sqA[:], sA[:], sA[:], op=A.mult)
                sqB1 = wp.tile([P, 256], mdt, tag="sqB1", name=f"sqB1_{w}")
                nc.vector.tensor_tensor(sqB1[:], sB1[:], sB1[:], op=A.mult)
                sqB2 = wp.tile([64, 128], mdt, tag="sqB2", name=f"sqB2_{w}")
                nc.vector.tensor_tensor(sqB2[:], sB2[:], sB2[:], op=A.mult)

                # channel-mix matmuls; base-0 vs base-64 sourced groups in
                # separate PSUM banks (disjoint PE row-groups run
                # concurrently and must not share a bank)
                d1s = (sA[64:128, 0:128], sA[0:64, 128:256],
                       sA[64:128, 128:256])
                d2s = (sA[0:64, 256:384], sA[64:128, 256:384],
                       sA[0:64, 384:512], sA[64:128, 384:512],
                       sB1[0:64, 0:128], sB1[64:128, 0:128],
                       sB1[0:64, 128:256], sB1[64:128, 128:256],
                       sB2[:, :])
                pLO = pmlp.tile([P, 384], f32, tag="pm", name=f"pLO{w}")
                pHI = pmlp.tile([P, 384], f32, tag="pm", name=f"pHI{w}")
                los = (d1s[1], d2s[0], d2s[2], d2s[4], d2s[6], d2s[8])
                his = (d1s[0], d1s[2], d2s[1], d2s[3], d2s[5], d2s[7])
                for j, dsrc in enumerate(los):
                    wmix = wl2_t if j else wl1_t
                    nc.tensor.matmul(pLO[:, 64 * j:64 * j + 64],
                                     lhsT=dsrc, rhs=wmix[0:64, :],
                                     start=True, stop=True)
                for j, dsrc in enumerate(his):
                    wmix = wl2_t if j >= 2 else wl1_t
                    nc.tensor.matmul(pHI[:, 64 * j:64 * j + 64],
                                     lhsT=dsrc, rhs=wmix[64:128, :],
                                     start=True, stop=True)
                sLO = wp.tile([P, 384], f32, tag="sLO", name=f"sLO{w}")
                nc.vector.tensor_copy(out=sLO[:], in_=pLO[:])
                sHI = wp.tile([P, 384], f32, tag="sHI", name=f"sHI{w}")
                nc.vector.tensor_copy(out=sHI[:], in_=pHI[:])

                # invariant folds on TensorE with 0/1 selection matrices
                pi1 = pmlp.tile([64, 128], f32, tag="pm", name=f"pi1_{w}")
                nc.tensor.matmul(pi1[:], lhsT=selhi_t[:], rhs=sqA[:, 0:128],
                                 start=True, stop=False)
                nc.tensor.matmul(pi1[:], lhsT=fold_t[:], rhs=sqA[:, 128:256],
                                 start=False, stop=True)
                v1 = wp.tile([64, 128], mdt, tag="v1", name=f"v1_{w}")
                nc.scalar.activation(v1[:], pi1[:], AF.Sqrt,
                                     bias=eps_t[:, 0:1])
                pi2 = pmlp.tile([64, 128], f32, tag="pm", name=f"pi2_{w}")
                nc.tensor.matmul(pi2[:], lhsT=fold_t[:], rhs=sqA[:, 256:384],
                                 start=True, stop=False)
                nc.tensor.matmul(pi2[:], lhsT=fold_t[:], rhs=sqA[:, 384:512],
                                 start=False, stop=False)
                nc.tensor.matmul(pi2[:], lhsT=fold_t[:], rhs=sqB1[:, 0:128],
                                 start=False, stop=False)
                nc.tensor.matmul(pi2[:], lhsT=fold_t[:], rhs=sqB1[:, 128:256],
                                 start=False, stop=False)
                nc.tensor.matmul(pi2[:], lhsT=id64_t[:], rhs=sqB2[:],
                                 start=False, stop=True)
                v2 = wp.tile([64, 128], mdt, tag="v2", name=f"v2_{w}")
                nc.scalar.activation(v2[:], pi2[:], AF.Sqrt,
                                     bias=eps_t[:, 0:1])
                return dict(sA=sA, v1=v1, v2=v2, sLO=sLO, sHI=sHI)

            def ph2b(w, ctx):
                """Serial scalar-update MLP + gating chain, one window
                behind ph2a so its cross-engine latencies are hidden."""
                if ctx is None:
                    return
                sA, v1, v2 = ctx["sA"], ctx["v1"], ctx["v2"]
                sLO, sHI = ctx["sLO"], ctx["sHI"]
                p1 = pmlp.tile([64, 128], f32, tag="pm", name=f"p1_{w}")
                nc.tensor.matmul(p1[:], lhsT=ws1t0_t[:], rhs=sA[0:64, 0:128],
                                 start=True, stop=False)
                nc.tensor.matmul(p1[:], lhsT=ws1t1_t[:], rhs=v1[:],
                                 start=False, stop=False)
                nc.tensor.matmul(p1[:], lhsT=ws1t2_t[:], rhs=v2[:],
                                 start=False, stop=True)
                hx = wp.tile([64, 128], mdt, tag="hx", name=f"hx{w}")
                nc.scalar.activation(hx[:], p1[:], AF.Identity,
                                     bias=bs1_t[:, 0:1])
                hs = wp.tile([64, 128], mdt, tag="hs", name=f"hs{w}")
                nc.scalar.activation(hs[:], p1[:], AF.Sigmoid,
                                     bias=bs1_t[:, 0:1])
                hT = wp.tile([64, 128], mdt, tag="hT", name=f"hT{w}")
                nc.vector.tensor_tensor(hT[:], hx[:], hs[:], op=A.mult)

                pd = pmlp.tile([64, 128], f32, tag="pm", name=f"pd{w}")
                nc.tensor.matmul(pd[:], lhsT=ws2_t[:], rhs=hT[:],
                                 start=True, stop=True)
                dh0T = wp.tile([64, 128], mdt, tag="dh0T", name=f"dh0T{w}")
                nc.scalar.activation(dh0T[:], pd[:], AF.Identity,
                                     bias=bs2c_t[:, 0:1])

                pn0 = pmlp.tile([P, 64], f32, tag="pm", name=f"pn0_{w}")
                nc.tensor.matmul(pn0[:], lhsT=hT[:], rhs=ws2_t[:],
                                 start=True, stop=True)
                oA = op.tile([P, FT], f32, tag="oA", name=f"oA{w}")
                nc.vector.tensor_tensor(oA[:, 0:64], pn0[:], bs2b_t[:],
                                        op=A.add)

                pg = pmlp.tile([64, 128], f32, tag="pm", name=f"pg{w}")
                nc.tensor.matmul(pg[:], lhsT=wg1_t[:], rhs=dh0T[:],
                                 start=True, stop=True)
                gx = wp.tile([64, 128], mdt, tag="gx", name=f"gx{w}")
                nc.scalar.activation(gx[:], pg[:], AF.Identity,
                                     bias=bg1_t[:, 0:1])
                gs = wp.tile([64, 128], mdt, tag="gs", name=f"gs{w}")
                nc.scalar.activation(gs[:], pg[:], AF.Sigmoid,
                                     bias=bg1_t[:, 0:1])
                hgT = wp.tile([64, 128], mdt, tag="hgT", name=f"hgT{w}")
                nc.vector.tensor_tensor(hgT[:], gx[:], gs[:], op=A.mult)

                ps = pmlp.tile([P, 128], f32, tag="pm", name=f"ps{w}")
                nc.tensor.matmul(ps[:], lhsT=hgT[:], rhs=wg2_t[:],
                                 start=True, stop=True)
                scal = wp.tile([P, 128], f32, tag="scal", name=f"scal{w}")
                nc.vector.tensor_tensor(scal[:], ps[:], bg2b_t[:], op=A.add)

                # gating: batched strided-block multiplies.
                # oA blocks k=0..11 at cols 64+64k; pHI holds blocks
                # (0,2 | 4,6,8,10), pLO holds (1 | 3,5,7,9,11).
                oAr = oA[:, 64:832].rearrange("p (k f) -> p k f", f=64)
                a1 = scal[:, 0:64].rearrange("p (o f) -> p o f", o=1)
                a2 = scal[:, 64:128].rearrange("p (o f) -> p o f", o=1)
                nc.vector.tensor_tensor(
                    oAr[:, 0:3:2, :],
                    sHI[:, 0:128].rearrange("p (k f) -> p k f", f=64),
                    a1.to_broadcast([P, 2, 64]), op=A.mult)
                nc.vector.tensor_tensor(
                    oAr[:, 4:11:2, :],
                    sHI[:, 128:384].rearrange("p (k f) -> p k f", f=64),
                    a2.to_broadcast([P, 4, 64]), op=A.mult)
                nc.vector.tensor_tensor(
                    oAr[:, 1:2, :],
                    sLO[:, 0:64].rearrange("p (k f) -> p k f", f=64),
                    a1.to_broadcast([P, 1, 64]), op=A.mult)
                nc.vector.tensor_tensor(
                    oAr[:, 3:12:2, :],
                    sLO[:, 64:384].rearrange("p (k f) -> p k f", f=64),
                    a2.to_broadcast([P, 5, 64]), op=A.mult)
                nc.sync.dma_start(out=outA[w * P:(w + 1) * P, :], in_=oA[:])

            # staggered pipeline: onehots two windows ahead so ph1
            # matmuls never wait on the DVE queue; ph1(w) | ph2a(w-1) |
            # ph2b(w-2)
            dens, ctxs, ohmap = {}, {}, {}
            for u in range(min(2, W)):
                ohmap[u] = ohbuild(u)
            for w in range(W):
                if w + 2 < W:
                    ohmap[w + 2] = ohbuild(w + 2)
                dens[w] = ph1(w, ohmap.pop(w))
                if w >= 1:
                    ctxs[w - 1] = ph2a(w - 1, *dens.pop(w - 1))
                if w >= 2:
                    ph2b(w - 2, ctxs.pop(w - 2))
            ctxs[W - 1] = ph2a(W - 1, *dens.pop(W - 1))
            for w in sorted(ctxs):
                ph2b(w, ctxs.pop(w))

    nc.compile()
    return nc


MSG_DTYPE = os.environ.get("KERNEL_MSG_DTYPE", "bf16")


def kernel(msg0, msg1, msg2, index, num_nodes,
           W_s1, b_s1, W_s2, b_s2, W_L1, W_L2, W_g1, b_g1, W_g2, b_g2):
    global LAST_EXEC_NS, LAST_RESULTS
    from concourse import bass_utils

    if MSG_DTYPE == "bf16":
        import ml_dtypes
        msg_np = ml_dtypes.bfloat16
    else:
        msg_np = np.float32
    E = int(np.asarray(index).shape[0])
    N = int(np.asarray(num_nodes))

    idx = np.asarray(index).astype(np.int64).ravel()
    perm = np.argsort(idx, kind="stable")
    sidx = idx[perm]

    starts, ncnt, ecnt = _pack_windows(idx, N)
    Wt = len(starts)
    Wc = -(-Wt // NCORES)           # windows per core
    Wpad = Wc * NCORES

    # slot layout
    E0 = np.concatenate(([0], np.cumsum(ecnt)))[:-1]
    win_of_edge = np.repeat(np.arange(Wt), ecnt)
    slot = win_of_edge * CAP + (np.arange(E) - E0[win_of_edge])

    lidx_g = np.full(Wpad * CAP, -1.0, np.float32)
    lidx_g[slot] = (sidx - starts[win_of_edge]).astype(np.float32)

    msgs_g = np.zeros((Wpad * CAP, FT), msg_np)
    m0 = np.asarray(msg0, np.float32)
    m1 = np.asarray(msg1, np.float32).reshape(E, 192)
    m2 = np.asarray(msg2, np.float32).reshape(E, 576)
    msgs_g[slot, 0:64] = m0[perm]
    msgs_g[slot, 64:256] = m1[perm]
    msgs_g[slot, 256:832] = m2[perm]
    # repack so row (w, p) = [edge slots w*CAP + k*128 + p for k in 0..TPW)
    # -> one descriptor per partition per window
    msgs_g = np.ascontiguousarray(
        msgs_g.reshape(Wpad, TPW, P, FT).transpose(0, 2, 1, 3)
    ).reshape(Wpad * P, TPW * FT)

    # weights / constants
    W_s1 = np.asarray(W_s1, np.float32)
    fold = np.zeros((128, 64), np.float32)
    fold[np.arange(128), np.arange(128) % 64] = 1.0
    selhi = np.zeros((128, 64), np.float32)
    selhi[np.arange(64) + 64, np.arange(64)] = 1.0
    cst = {
        "iota": np.ascontiguousarray(
            np.broadcast_to(np.arange(P, dtype=np.float32), (P, P))
        ).astype(msg_np),
        "ws1t0": np.ascontiguousarray(W_s1.T[0:64]).astype(msg_np),
        "ws1t1": np.ascontiguousarray(W_s1.T[64:128]).astype(msg_np),
        "ws1t2": np.ascontiguousarray(W_s1.T[128:192]).astype(msg_np),
        "fold": fold.astype(msg_np),
        "selhi": selhi.astype(msg_np),
        "id64": np.eye(64, dtype=np.float32).astype(msg_np),
        "ws2": np.ascontiguousarray(
            np.asarray(W_s2, np.float32).T).astype(msg_np),
        "wg1": np.ascontiguousarray(
            np.asarray(W_g1, np.float32).T).astype(msg_np),
        "wg2": np.ascontiguousarray(
            np.asarray(W_g2, np.float32).T).astype(msg_np),
        "wl1": np.ascontiguousarray(
            np.vstack([np.asarray(W_L1, np.float32).T] * 2)).astype(msg_np),
        "wl2": np.ascontiguousarray(
            np.vstack([np.asarray(W_L2, np.float32).T] * 2)).astype(msg_np),
        "bs1": np.asarray(b_s1, np.float32).reshape(64, 1),
        "bg1": np.asarray(b_g1, np.float32).reshape(64, 1),
        "bs2c": np.asarray(b_s2, np.float32).reshape(64, 1),
        "bs2b": np.ascontiguousarray(
            np.broadcast_to(np.asarray(b_s2, np.float32), (P, 64))),
        "bg2b": np.ascontiguousarray(
            np.broadcast_to(np.asarray(b_g2, np.float32), (P, 128))),
        "eps": np.full((64, 1), 1e-8, np.float32),
    }

    nc = _build_program(Wc, msg_np)

    in_maps = []
    for c in range(NCORES):
        lo, hi = c * Wc * CAP, (c + 1) * Wc * CAP
        lidx_c = np.ascontiguousarray(
            lidx_g[lo:hi].reshape(Wc * TPW, P).T)
        in_maps.append({"msgs": msgs_g[c * Wc * P:(c + 1) * Wc * P],
                        "lidx": lidx_c, **cst})

    trace = os.environ.get("KERNEL_PROFILE", "0") == "1"
    if trace:
        _install_ntff_hook()
    res = bass_utils.run_bass_kernel_spmd(
        nc, in_maps, core_ids=list(range(NCORES)), trace=trace)
    LAST_RESULTS = res
    LAST_EXEC_NS = res.exec_time_ns

    # unpack outputs
    delta0 = np.empty((N, 64), np.float32)
    delta1 = np.empty((N, 192), np.float32)
    delta2 = np.empty((N, 576), np.float32)
    win_of_node = np.repeat(np.arange(Wt), ncnt)
    pos = np.arange(N) - starts[win_of_node]
    rows = (win_of_node % Wc) * P + pos
    cores = win_of_node // Wc
    for c in range(NCORES):
        m = cores == c
        if not m.any():
            continue
        r = rows[m]
        oa = np.asarray(res.results[c]["outA"])
        delta0[m] = oa[r, 0:64]
        delta1[m] = oa[r, 64:256]
        delta2[m] = oa[r, 256:832]

    return (delta0, delta1.reshape(N, 3, 64), delta2.reshape(N, 3, 3, 64))
